# revision 32
# baseline (speedup 1.0000x reference)
"""Trainium2 Bass kernel for nn_ModelNew_3556232922055 (dense_cnn).

Semantics (per image):
  y8[ci]   = conv2d_valid(x, weight[:8]) + bias[:8]          (8,126,126)
  acc[co]  = max over (ci,kh,kw) of 2*W[co,ci,kh,kw]*y8[ci,h+kh,w+kw]
             (out-of-range taps excluded at the bottom/right borders)
  out      = min over co of acc                              (1,126,126)

Sharding: data-parallel over batch, 1 image per NeuronCore (8 cores).

Device mapping per core (v5):
  - host-built im2col X72 [72, 16128] bf16, streamed in 4 quarter DMAs
  - conv: k=72 bf16 matmuls -> PSUM f32 -> ACT evac (+bias) into
    Y8 [128, 16128] bf16 where partition p = ci*16 + r holds y8[ci]
  - step 2 in row-bands (42,42,21,21); per band, per tap, per co-half
    (A: co 0-15 on partition ci*16+co, B: co 16-31):
      product = scal[p,t] * y8[ci, pix+off]
      kw=0 taps: DVE tensor_scalar on full-width flat views (4x mode)
      kw!=0 taps: ACT mul on [p, nh, nw] views (col masking)
    then DVE tensor_tensor(max) accumulates into pacc (2x mode)
  - per band reduce: one DMA-XBAR transpose per half lands
    pt[w, c, (ci,colo)] in SBUF; DVE folds (flat contiguous views)
    max over ci then min over co -> OUT[w, h]; finally PE transpose ->
    DMA out (126,126) f32.
"""

import numpy as np
from contextlib import ExitStack

import concourse.bass as bass
import concourse.tile as tile
from concourse import bacc, mybir
from concourse import masks
from concourse.bass_utils import run_bass_kernel_spmd

F32 = mybir.dt.float32
BF16 = mybir.dt.bfloat16

H = W = 128
CIN = 8
COUT = 32
K = 3
OH = OW = 126
NPIX = H * OH          # 16128 flat pixels (h*128+w), h<126
NCORES = 8
CHUNK = 504            # conv free-dim chunk (<=512, 32*504=16128)
BANDS = [(0, 21), (21, 63), (63, 105), (105, 126)]
BH = 42

TAPS = [(kh, kw) for kh in range(K) for kw in range(K)]


def _r3(t, h0, nh, w0, nw):
    """3D region view [128, nh, nw] of a [128, NPIX] tile at rows h0, cols w0."""
    return t[:].rearrange("p (h w) -> p h w", w=W)[:, h0 : h0 + nh, w0 : w0 + nw]


def _flat(t, h0, nh):
    """Flat contiguous view [128, nh*128] of rows [h0, h0+nh)."""
    return t[:, h0 * W : (h0 + nh) * W]


def build_program():
    nc = bacc.Bacc()

    # x72: host-built im2col, x72[(kh*3+kw)*8+ci, pix] = x[ci, pix+kh*128+kw]
    x_d = nc.declare_dram_parameter("x72", [72, NPIX], BF16, isOutput=False)
    # consts f32: [:, 0:18] scal, [:, 18:19] bias128
    c_d = nc.declare_dram_parameter("consts", [128, 19], F32, isOutput=False)
    # conv weights, replicated: w1[t, ci*16+r] = weight[ci, t_ci, t_kh, t_kw]
    w_d = nc.declare_dram_parameter("w1", [72, 128], BF16, isOutput=False)
    out_d = nc.declare_dram_parameter("out", [OH, OW], F32, isOutput=True)

    with ExitStack() as ctx:
        tc = ctx.enter_context(tile.TileContext(nc))

        consts = ctx.enter_context(tc.tile_pool(name="consts", bufs=1))
        big = ctx.enter_context(tc.tile_pool(name="big", bufs=1))

        constst = consts.tile([128, 19], F32)
        nc.sync.dma_start(constst[:], c_d[:])
        scalt = constst[:, 0:18]
        biast = constst[:, 18:19]
        w1t = consts.tile([72, 128], BF16)
        nc.sync.dma_start(w1t[:], w_d[:])

        y8 = big.tile([128, NPIX], BF16)
        paccs = []
        for half in range(2):
            pacc = big.tile([128, NPIX], BF16, tag=f"pacc{half}")
            paccs.append(pacc)
        outt = big.tile([128, 128], BF16)  # OUT[w, h] (cols 126/127 junk)

        ppool = ctx.enter_context(tc.tile_pool(name="ppool", bufs=6))
        redpool = ctx.enter_context(tc.tile_pool(name="redpool", bufs=1))
        xp = ctx.enter_context(tc.tile_pool(name="xp", bufs=2))
        psum = ctx.enter_context(tc.tile_pool(name="psum", bufs=3, space="PSUM"))

        # --- load + conv: y8[p = ci*16+r] = y8[ci], bf16 ---
        QRT = NPIX // 4  # 4032 = 1024*3 + 960
        for h in range(4):
            xh = xp.tile([72, QRT], BF16, tag="xh")
            nc.sync.dma_start(out=xh[:], in_=x_d[:, h * QRT : (h + 1) * QRT])
            # 512-col matmul slices (bank-aligned), two banks per PSUM
            # tile, one merged ACT evac per tile
            n0 = 0
            for cn in (1024, 1024, 1024, 960):
                ps = psum.tile([128, 1024], F32, tag="convps")
                for j in range(0, cn, 512):
                    jn = min(512, cn - j)
                    nc.tensor.matmul(
                        ps[:, j : j + jn], lhsT=w1t[:],
                        rhs=xh[:, n0 + j : n0 + j + jn],
                        start=True, stop=True,
                    )
                nc.scalar.activation(
                    y8[:, h * QRT + n0 : h * QRT + n0 + cn], ps[:, 0:cn],
                    mybir.ActivationFunctionType.Identity,
                    bias=biast, scale=1.0,
                )
                n0 += cn

        mx = mybir.AluOpType.max
        mn = mybir.AluOpType.min
        mu = mybir.AluOpType.mult

        def reduce_band(h0, bh):
            """Transpose band rows [h0, h0+bh) of both pacc halves via the
            DMA XBAR, then fold max over ci, min over co into outt[:, h0:].

            pt element (p, s*bh*128 + c*128 + j) = pacc_s[j, (h0+c)*128+p],
            j = ci*16 + colo. All folds use flat 3D views: cols 0:64 of each
            128-block are ci 0..3, etc.
            """
            pt = redpool.tile([128, 2 * BH * W], BF16, tag="PT")
            tt = nc.vector.tensor_tensor
            for half in range(2):
                sect = pt[:, half * bh * W : (half + 1) * bh * W]
                dst = sect.rearrange("p (c j) -> p c j", j=128)
                nc.sync.dma_start_transpose(dst, _flat(paccs[half], h0, bh))
                # first ci fold per half: starts as soon as its transpose lands
                f = sect.rearrange("p (c j) -> p c j", j=128)
                tt(f[:, :, 0:64], f[:, :, 0:64], f[:, :, 64:128], mx)  # ci 03|47
            # remaining ci folds over both halves at once
            fa = pt[:, 0 : 2 * bh * W].rearrange("p (sc j) -> p sc j", j=128)
            tt(fa[:, :, 0:32], fa[:, :, 0:32], fa[:, :, 32:64], mx)    # ci 01|23
            tt(fa[:, :, 0:16], fa[:, :, 0:16], fa[:, :, 16:32], mx)    # ci 0|1
            g0 = pt[:, 0 : bh * W].rearrange("p (c j) -> p c j", j=128)
            g1 = pt[:, bh * W : 2 * bh * W].rearrange("p (c j) -> p c j", j=128)
            w2 = g0[:, :, 0:16]
            tt(w2, w2, g1[:, :, 0:16], mn)                 # co half A|B
            tt(w2[:, :, 0:8], w2[:, :, 0:8], w2[:, :, 8:16], mn)
            tt(w2[:, :, 0:4], w2[:, :, 0:4], w2[:, :, 4:8], mn)
            tt(w2[:, :, 0:2], w2[:, :, 0:2], w2[:, :, 2:4], mn)
            tt(outt[:, h0 : h0 + bh], w2[:, :, 0:1], w2[:, :, 1:2], mn)

        # --- step 2, banded; reduce band k while band k+1 computes ---
        # DVE-fed taps (kw=0) first so ACT builds product lookahead.
        # Band 0 runs while ACT is busy with conv evacs, so it is DVE-heavy.
        DVE_TAPS = {0: {3, 6, 1, 2, 4, 5}, 1: {3, 6}, 2: {3, 6}, 3: {3, 6}}
        for bi, (h0, h1) in enumerate(BANDS):
            bh = h1 - h0
            # tap 0 covers the full band (incl. junk cols 126/127), flat.
            for half in range(2):
                nc.vector.tensor_scalar(
                    _flat(paccs[half], h0, bh), _flat(y8, h0, bh),
                    scalt[:, half * 9 : half * 9 + 1], None, mu,
                )
            dve_taps = DVE_TAPS[bi]
            for t in [3, 6, 1, 2, 4, 5, 7, 8]:
                kh, kw = TAPS[t]
                nh = min(h1, OH - kh) - h0
                nw = W if kw == 0 else OW - kw
                prods = []
                for half in range(2):
                    p = ppool.tile([128, BH * W], BF16, tag="P")
                    sc = scalt[:, half * 9 + t : half * 9 + t + 1]
                    if kw == 0:
                        # full-width flat: contiguous, DVE 4x; junk cols
                        # 126/127 of each row are dropped by the reduce.
                        p3 = p[:, 0 : nh * W]
                        src = _flat(y8, h0 + kh, nh)
                        nc.vector.tensor_scalar(p3, src, sc, None, mu)
                    else:
                        p3 = p[:].rearrange("p (h w) -> p h w", w=W)[
                            :, 0:nh, 0:nw
                        ]
                        src = _r3(y8, h0 + kh, nh, kw, nw)
                        if t in dve_taps:
                            nc.vector.tensor_scalar(p3, src, sc, None, mu)
                        else:
                            nc.scalar.mul(p3, src, sc)
                    prods.append(p3)
                for half in range(2):
                    if kw == 0:
                        acc3 = _flat(paccs[half], h0, nh)
                    else:
                        acc3 = _r3(paccs[half], h0, nh, 0, nw)
                    nc.vector.tensor_tensor(acc3, acc3, prods[half], mx)
            reduce_band(h0, bh)

        # transpose OUT[w,h] -> [h,w] via the DMA XBAR and write out
        outh = consts.tile([128, 128], BF16)
        nc.sync.dma_start_transpose(outh[:], outt[:])
        res = consts.tile([128, 128], F32)
        nc.scalar.copy(res[0:OH, :], outh[0:OH, :])
        nc.sync.dma_start(out_d[:, :], res[0:OH, 0:OW])

    nc.compile()
    return nc


def host_tiles(weight, bias):
    weight = np.asarray(weight, np.float32)
    bias = np.asarray(bias, np.float32)
    w1rep = np.zeros((72, 128), np.float32)
    for kh in range(K):
        for kw in range(K):
            for ci_in in range(CIN):
                t = (kh * K + kw) * CIN + ci_in
                for ci_out in range(CIN):
                    w1rep[t, ci_out * 16 : ci_out * 16 + 16] = weight[
                        ci_out, ci_in, kh, kw
                    ]
    bias128 = np.repeat(bias[:CIN], 16).astype(np.float32).reshape(128, 1)
    scal = np.zeros((128, 18), np.float32)
    for p in range(128):
        ci = p // 16
        co_lo = p % 16
        for half in range(2):
            co = co_lo + 16 * half
            for t, (kh, kw) in enumerate(TAPS):
                scal[p, half * 9 + t] = 2.0 * weight[co, ci, kh, kw]
    consts = np.zeros((128, 19), np.float32)
    consts[:, 0:18] = scal
    consts[:, 18:19] = bias128
    return consts, w1rep


def im2col_host(xb):
    """xb: (8,128,128) f32 -> (72, NPIX) bf16 with junk tail cols zeroed."""
    import ml_dtypes

    x72 = np.zeros((72, NPIX), np.float32)
    L = NPIX - 2
    flat = xb.reshape(-1)
    for kh in range(K):
        for kw in range(K):
            for ci in range(CIN):
                t = (kh * K + kw) * CIN + ci
                off = kh * W + kw
                x72[t, :L] = flat[ci * H * W + off : ci * H * W + off + L]
    return x72.astype(ml_dtypes.bfloat16)


_CACHE = {}


def _get_program():
    if "nc" not in _CACHE:
        _CACHE["nc"] = build_program()
    return _CACHE["nc"]


def run_spmd(x, weight, bias, **kw):
    import ml_dtypes

    x = np.ascontiguousarray(np.asarray(x, np.float32))
    consts, w1rep = host_tiles(weight, bias)
    w1_bf16 = w1rep.astype(ml_dtypes.bfloat16)
    nc = _get_program()
    in_maps = [
        {"x72": im2col_host(x[b]), "consts": consts, "w1": w1_bf16}
        for b in range(NCORES)
    ]
    res = run_bass_kernel_spmd(nc, in_maps, list(range(NCORES)), **kw)
    out = np.stack([res.results[b]["out"] for b in range(NCORES)])
    return out[:, None, :, :].astype(np.float32), res


def kernel(x, weight, bias):
    out, _ = run_spmd(x, weight, bias)
    return out


if __name__ == "__main__":
    rng = np.random.default_rng(0)
    x = rng.standard_normal((8, CIN, H, W), dtype=np.float32)
    wt = rng.uniform(-0.1, 0.1, (COUT, CIN, K, K)).astype(np.float32)
    bs = rng.uniform(-0.1, 0.1, COUT).astype(np.float32)
    print(kernel(x, wt, bs).shape)


# revision 37
# speedup vs baseline: 1.0134x; 1.0134x over previous
"""Trainium2 Bass kernel for nn_ModelNew_3556232922055 (dense_cnn).

Semantics (per image):
  y8[ci]   = conv2d_valid(x, weight[:8]) + bias[:8]          (8,126,126)
  acc[co]  = max over (ci,kh,kw) of 2*W[co,ci,kh,kw]*y8[ci,h+kh,w+kw]
             (out-of-range taps excluded at the bottom/right borders)
  out      = min over co of acc                              (1,126,126)

Sharding: data-parallel over batch, 1 image per NeuronCore (8 cores).

Device mapping per core (v5):
  - host-built im2col X72 [72, 16128] bf16, streamed in 4 quarter DMAs
  - conv: k=72 bf16 matmuls -> PSUM f32 -> ACT evac (+bias) into
    Y8 [128, 16128] bf16 where partition p = ci*16 + r holds y8[ci]
  - step 2 in row-bands (42,42,21,21); per band, per tap, per co-half
    (A: co 0-15 on partition ci*16+co, B: co 16-31):
      product = scal[p,t] * y8[ci, pix+off]
      kw=0 taps: DVE tensor_scalar on full-width flat views (4x mode)
      kw!=0 taps: ACT mul on [p, nh, nw] views (col masking)
    then DVE tensor_tensor(max) accumulates into pacc (2x mode)
  - per band reduce: one DMA-XBAR transpose per half lands
    pt[w, c, (ci,colo)] in SBUF; DVE folds (flat contiguous views)
    max over ci then min over co -> OUT[w, h]; finally PE transpose ->
    DMA out (126,126) f32.
"""

import numpy as np
from contextlib import ExitStack

import concourse.bass as bass
import concourse.tile as tile
from concourse import bacc, mybir
from concourse import masks
from concourse.bass_utils import run_bass_kernel_spmd

F32 = mybir.dt.float32
BF16 = mybir.dt.bfloat16

H = W = 128
CIN = 8
COUT = 32
K = 3
OH = OW = 126
NPIX = H * OH          # 16128 flat pixels (h*128+w), h<126
NCORES = 8
CHUNK = 504            # conv free-dim chunk (<=512, 32*504=16128)
BANDS = [(0, 21), (21, 63), (63, 105), (105, 126)]
BH = 42

TAPS = [(kh, kw) for kh in range(K) for kw in range(K)]


def _r3(t, h0, nh, w0, nw):
    """3D region view [128, nh, nw] of a [128, NPIX] tile at rows h0, cols w0."""
    return t[:].rearrange("p (h w) -> p h w", w=W)[:, h0 : h0 + nh, w0 : w0 + nw]


def _flat(t, h0, nh):
    """Flat contiguous view [128, nh*128] of rows [h0, h0+nh)."""
    return t[:, h0 * W : (h0 + nh) * W]


def build_program():
    nc = bacc.Bacc()

    # x72: host-built im2col, x72[(kh*3+kw)*8+ci, pix] = x[ci, pix+kh*128+kw]
    x_d = nc.declare_dram_parameter("x72", [72, NPIX], BF16, isOutput=False)
    # consts f32: [:, 0:18] scal, [:, 18:19] bias128
    c_d = nc.declare_dram_parameter("consts", [128, 19], F32, isOutput=False)
    # conv weights, replicated: w1[t, ci*16+r] = weight[ci, t_ci, t_kh, t_kw]
    w_d = nc.declare_dram_parameter("w1", [72, 128], BF16, isOutput=False)
    out_d = nc.declare_dram_parameter("out", [OH, OW], F32, isOutput=True)

    with ExitStack() as ctx:
        tc = ctx.enter_context(tile.TileContext(nc))

        consts = ctx.enter_context(tc.tile_pool(name="consts", bufs=1))
        big = ctx.enter_context(tc.tile_pool(name="big", bufs=1))

        constst = consts.tile([128, 19], F32)
        nc.sync.dma_start(constst[:], c_d[:])
        scalt = constst[:, 0:18]
        biast = constst[:, 18:19]
        w1t = consts.tile([72, 128], BF16)
        nc.sync.dma_start(w1t[:], w_d[:])

        y8 = big.tile([128, NPIX], BF16)
        paccs = []
        for half in range(2):
            pacc = big.tile([128, NPIX], BF16, tag=f"pacc{half}")
            paccs.append(pacc)
        outt = big.tile([128, 128], BF16)  # OUT[w, h] (cols 126/127 junk)

        ppool = ctx.enter_context(tc.tile_pool(name="ppool", bufs=7))
        redpool = ctx.enter_context(tc.tile_pool(name="redpool", bufs=1))
        xp = ctx.enter_context(tc.tile_pool(name="xp", bufs=2))
        psum = ctx.enter_context(tc.tile_pool(name="psum", bufs=3, space="PSUM"))

        # --- load + conv: y8[p = ci*16+r] = y8[ci], bf16 ---
        QRT = NPIX // 4  # 4032 = 1024*3 + 960
        for h in range(4):
            xh = xp.tile([72, QRT], BF16, tag="xh")
            nc.sync.dma_start(out=xh[:], in_=x_d[:, h * QRT : (h + 1) * QRT])
            # 512-col matmul slices (bank-aligned), two banks per PSUM
            # tile, one merged ACT evac per tile
            n0 = 0
            for cn in (1024, 1024, 1024, 960):
                ps = psum.tile([128, 1024], F32, tag="convps")
                for j in range(0, cn, 512):
                    jn = min(512, cn - j)
                    nc.tensor.matmul(
                        ps[:, j : j + jn], lhsT=w1t[:],
                        rhs=xh[:, n0 + j : n0 + j + jn],
                        start=True, stop=True,
                    )
                nc.scalar.activation(
                    y8[:, h * QRT + n0 : h * QRT + n0 + cn], ps[:, 0:cn],
                    mybir.ActivationFunctionType.Identity,
                    bias=biast, scale=1.0,
                )
                n0 += cn

        mx = mybir.AluOpType.max
        mn = mybir.AluOpType.min
        mu = mybir.AluOpType.mult

        def reduce_band(h0, bh, last=False):
            """Transpose band rows [h0, h0+bh) of both pacc halves via the
            DMA XBAR, then fold max over ci, min over co into outt[:, h0:].

            pt element (p, s*bh*128 + c*128 + j) = pacc_s[j, (h0+c)*128+p],
            j = ci*16 + colo. All folds use flat 3D views: cols 0:64 of each
            128-block are ci 0..3, etc.
            """
            pt = redpool.tile([128, 2 * BH * W], BF16, tag="PT")
            tt = nc.vector.tensor_tensor
            for half in range(2):
                sect = pt[:, half * bh * W : (half + 1) * bh * W]
                dst = sect.rearrange("p (c j) -> p c j", j=128)
                # for the last band ACT is idle: issue half 1 there so the
                # two transposes run on separate queues in the tail
                qeng = nc.scalar if (last and half == 1) else nc.sync
                qeng.dma_start_transpose(dst, _flat(paccs[half], h0, bh))
                # first ci fold per half: starts as soon as its transpose lands
                f = sect.rearrange("p (c j) -> p c j", j=128)
                tt(f[:, :, 0:64], f[:, :, 0:64], f[:, :, 64:128], mx)  # ci 03|47
            # remaining ci folds over both halves at once
            fa = pt[:, 0 : 2 * bh * W].rearrange("p (sc j) -> p sc j", j=128)
            tt(fa[:, :, 0:32], fa[:, :, 0:32], fa[:, :, 32:64], mx)    # ci 01|23
            tt(fa[:, :, 0:16], fa[:, :, 0:16], fa[:, :, 16:32], mx)    # ci 0|1
            g0 = pt[:, 0 : bh * W].rearrange("p (c j) -> p c j", j=128)
            g1 = pt[:, bh * W : 2 * bh * W].rearrange("p (c j) -> p c j", j=128)
            w2 = g0[:, :, 0:16]
            tt(w2, w2, g1[:, :, 0:16], mn)                 # co half A|B
            tt(w2[:, :, 0:8], w2[:, :, 0:8], w2[:, :, 8:16], mn)
            tt(w2[:, :, 0:4], w2[:, :, 0:4], w2[:, :, 4:8], mn)
            tt(w2[:, :, 0:2], w2[:, :, 0:2], w2[:, :, 2:4], mn)
            tt(outt[:, h0 : h0 + bh], w2[:, :, 0:1], w2[:, :, 1:2], mn)

        # --- step 2, banded; reduce band k while band k+1 computes ---
        # DVE-fed taps (kw=0) first so ACT builds product lookahead.
        # Band 0 runs while ACT is busy with conv evacs, so it is DVE-heavy.
        DVE_TAPS = {0: {3, 6, 1, 4}, 1: {3, 6}, 2: {3, 6}, 3: {3, 6}}
        for bi, (h0, h1) in enumerate(BANDS):
            bh = h1 - h0
            # tap 0 covers the full band (incl. junk cols 126/127), flat.
            for half in range(2):
                nc.vector.tensor_scalar(
                    _flat(paccs[half], h0, bh), _flat(y8, h0, bh),
                    scalt[:, half * 9 : half * 9 + 1], None, mu,
                )
            dve_taps = DVE_TAPS[bi]
            for t in [3, 6, 1, 2, 4, 5, 7, 8]:
                kh, kw = TAPS[t]
                nh = min(h1, OH - kh) - h0
                nw = W if kw == 0 else OW - kw
                prods = []
                for half in range(2):
                    p = ppool.tile([128, BH * W], BF16, tag="P")
                    sc = scalt[:, half * 9 + t : half * 9 + t + 1]
                    if kw == 0:
                        # full-width flat: contiguous, DVE 4x; junk cols
                        # 126/127 of each row are dropped by the reduce.
                        p3 = p[:, 0 : nh * W]
                        src = _flat(y8, h0 + kh, nh)
                        nc.vector.tensor_scalar(p3, src, sc, None, mu)
                    else:
                        p3 = p[:].rearrange("p (h w) -> p h w", w=W)[
                            :, 0:nh, 0:nw
                        ]
                        src = _r3(y8, h0 + kh, nh, kw, nw)
                        if t in dve_taps:
                            nc.vector.tensor_scalar(p3, src, sc, None, mu)
                        else:
                            nc.scalar.mul(p3, src, sc)
                    prods.append(p3)
                for half in range(2):
                    if kw == 0:
                        acc3 = _flat(paccs[half], h0, nh)
                    else:
                        acc3 = _r3(paccs[half], h0, nh, 0, nw)
                    nc.vector.tensor_tensor(acc3, acc3, prods[half], mx)
            reduce_band(h0, bh, last=(bi == len(BANDS) - 1))

        # transpose OUT[w,h] -> [h,w] via the DMA XBAR and write out
        outh = consts.tile([128, 128], BF16)
        nc.sync.dma_start_transpose(outh[:], outt[:])
        res = consts.tile([128, 128], F32)
        nc.scalar.copy(res[0:OH, :], outh[0:OH, :])
        nc.sync.dma_start(out_d[:, :], res[0:OH, 0:OW])

    nc.compile()
    return nc


def host_tiles(weight, bias):
    weight = np.asarray(weight, np.float32)
    bias = np.asarray(bias, np.float32)
    w1rep = np.zeros((72, 128), np.float32)
    for kh in range(K):
        for kw in range(K):
            for ci_in in range(CIN):
                t = (kh * K + kw) * CIN + ci_in
                for ci_out in range(CIN):
                    w1rep[t, ci_out * 16 : ci_out * 16 + 16] = weight[
                        ci_out, ci_in, kh, kw
                    ]
    bias128 = np.repeat(bias[:CIN], 16).astype(np.float32).reshape(128, 1)
    scal = np.zeros((128, 18), np.float32)
    for p in range(128):
        ci = p // 16
        co_lo = p % 16
        for half in range(2):
            co = co_lo + 16 * half
            for t, (kh, kw) in enumerate(TAPS):
                scal[p, half * 9 + t] = 2.0 * weight[co, ci, kh, kw]
    consts = np.zeros((128, 19), np.float32)
    consts[:, 0:18] = scal
    consts[:, 18:19] = bias128
    return consts, w1rep


def im2col_host(xb):
    """xb: (8,128,128) f32 -> (72, NPIX) bf16 with junk tail cols zeroed."""
    import ml_dtypes

    x72 = np.zeros((72, NPIX), np.float32)
    L = NPIX - 2
    flat = xb.reshape(-1)
    for kh in range(K):
        for kw in range(K):
            for ci in range(CIN):
                t = (kh * K + kw) * CIN + ci
                off = kh * W + kw
                x72[t, :L] = flat[ci * H * W + off : ci * H * W + off + L]
    return x72.astype(ml_dtypes.bfloat16)


_CACHE = {}


def _get_program():
    if "nc" not in _CACHE:
        _CACHE["nc"] = build_program()
    return _CACHE["nc"]


def run_spmd(x, weight, bias, **kw):
    import ml_dtypes

    x = np.ascontiguousarray(np.asarray(x, np.float32))
    consts, w1rep = host_tiles(weight, bias)
    w1_bf16 = w1rep.astype(ml_dtypes.bfloat16)
    nc = _get_program()
    in_maps = [
        {"x72": im2col_host(x[b]), "consts": consts, "w1": w1_bf16}
        for b in range(NCORES)
    ]
    res = run_bass_kernel_spmd(nc, in_maps, list(range(NCORES)), **kw)
    out = np.stack([res.results[b]["out"] for b in range(NCORES)])
    return out[:, None, :, :].astype(np.float32), res


def kernel(x, weight, bias):
    out, _ = run_spmd(x, weight, bias)
    return out


if __name__ == "__main__":
    rng = np.random.default_rng(0)
    x = rng.standard_normal((8, CIN, H, W), dtype=np.float32)
    wt = rng.uniform(-0.1, 0.1, (COUT, CIN, K, K)).astype(np.float32)
    bs = rng.uniform(-0.1, 0.1, COUT).astype(np.float32)
    print(kernel(x, wt, bs).shape)
